# revision 3
# baseline (speedup 1.0000x reference)
"""Trainium2 Bass kernel V3 for sparse 3D voxel convolution (e3nn-style, 5^3 taps).

Sharding: data-parallel over the N=200000 sparse voxels, sorted by x-plane and
split into 8 contiguous slabs of 25000 destination voxels; each core holds a
local bf16 feature table (slab + halo, <32k rows, int16 gather ids).

Single-phase per-tap pipeline per core:
  - center tap + residual: the slab's features live transposed in SBUF
    ([feat, dst] bf16 strip); one matmul per 128-dst block against the
    center kernel accumulates in PSUM and stores contiguous f32 output rows.
  - 124 sparse taps, one gather + one scatter-add per tap (pair lists padded
    to the max count over cores; pads gather row 0 and scatter into a trash
    row): bf16 dma_gather (256B rows) -> PE transpose (bf16 identity) ->
    matmul against the tap kernel -> bf16 dma_scatter_add (160B payload,
    256B row pitch) into one of 4 bf16 tables, table == queue so RMW stays
    ordered. Destinations are unique within a tap, so no updates are lost.

Host sums out + the 4 tables (the conv part is ~1% of the residual, bf16
accumulation is far inside the tolerance).
"""

import sys
import types

import numpy as np
import ml_dtypes

NRB = 8
RAD = 2.5
GRID = 192
N = 200000
DIM = 80
ALPHA = 1.0 / np.sqrt(48.0)
N_CORES = 8
N_LOC = N // N_CORES            # 25000 dst voxels per core
NBLK = (N_LOC + 127) // 128     # 196 out blocks
NT = NBLK * 128                 # 25088 out rows
TRASH = NT                      # scatter pad row (tables have NT+1... rows)
NTT = NT + 128                  # table rows incl trash
SUB = 4                         # columns per PSUM tile
N_TBL = 4                       # scatter tables == queues

_ax = np.arange(-2.0, 3.0, dtype=np.float32)
LATTICE = np.stack(np.meshgrid(_ax, _ax, _ax, indexing="ij"), -1)
PERM = np.arange(125).reshape(5, 5, 5).transpose(2, 1, 0).reshape(-1)
OFFS = LATTICE.reshape(-1, 3).astype(np.int32)[PERM]
CENTER_TAP = 62
TAPS = [t for t in range(125) if t != CENTER_TAP]


def _radial_emb():
    d = np.linalg.norm(LATTICE, axis=-1)
    centers = np.linspace(0.0, RAD, NRB + 2)[1:-1]
    step = centers[1] - centers[0]
    t = (d[..., None] - centers) / step
    inside = np.abs(t) < 1.0
    safe = np.where(inside, 1.0 - t * t, 1.0)
    return (1.14136 * np.exp(2.0) * np.where(inside, np.exp(-2.0 / safe), 0.0)).astype(
        np.float32
    )


EMB = _radial_emb().reshape(-1, NRB)


def _sph():
    n = np.linalg.norm(LATTICE, axis=-1, keepdims=True)
    u = np.where(n > 0, LATTICE / np.maximum(n, 1e-9), 0.0)
    return np.concatenate([np.ones_like(n), np.sqrt(3.0) * u], -1).astype(np.float32)


SH = _sph().reshape(-1, 4)


def make_kernel_np(weight):
    w = (EMB @ weight.astype(np.float32)) / 125.0
    w1 = w[:, :1024].reshape(125, 32, 32)
    w2 = w[:, 1024:1536].reshape(125, 32, 16)
    w3 = w[:, 1536:1792].reshape(125, 16, 16)
    w4 = w[:, 1792:].reshape(125, 16, 32)
    s0 = SH[:, 0]
    v = SH[:, 1:]
    eye3 = np.eye(3, dtype=w.dtype)
    K00 = ALPHA * w1 * s0[:, None, None]
    K01 = ALPHA * np.einsum("pik,pm->pikm", w2, v).reshape(125, 32, 48)
    K11 = ALPHA * np.einsum(
        "pik,mn->pimkn", w3 * s0[:, None, None], eye3
    ).reshape(125, 48, 48)
    K10 = (ALPHA / np.sqrt(3.0)) * np.einsum("pik,pm->pimk", w4, v).reshape(125, 48, 32)
    K = np.concatenate(
        [np.concatenate([K00, K01], 2), np.concatenate([K10, K11], 2)], 1
    )
    return K[PERM]


def w_sc_embed(w_sc0, w_sc1):
    W = np.zeros((80, 80), np.float32)
    W[:32, :32] = w_sc0 / np.sqrt(32.0)
    blk = np.zeros((48, 48), np.float32)
    for m in range(3):
        blk[m::3, m::3] = w_sc1 / np.sqrt(16.0)
    W[32:, 32:] = blk
    return W


def build_pairs(coords):
    idx_vol = np.full(GRID * GRID * GRID, -1, np.int32)
    lin = (coords[:, 0].astype(np.int64) * GRID + coords[:, 1]) * GRID + coords[:, 2]
    idx_vol[lin] = np.arange(N, dtype=np.int32)
    all_i = np.arange(N, dtype=np.int32)
    dsts, srcs = [], []
    for t in range(125):
        if t == CENTER_TAP:
            dsts.append(None)
            srcs.append(None)
            continue
        c = coords + OFFS[t]
        ok = np.all((c >= 0) & (c < GRID), axis=1)
        cl = (c[:, 0].astype(np.int64) * GRID + c[:, 1]) * GRID + c[:, 2]
        cl = np.clip(cl, 0, GRID**3 - 1)
        nb = idx_vol[cl]
        valid = ok & (nb >= 0)
        dsts.append(all_i[valid])
        srcs.append(nb[valid])
    return dsts, srcs


def wrap16(a):
    """Token stream [n] -> [128, n//16] int16 (16-partition wrap, 8x replicated)."""
    n = a.shape[0]
    w = a.reshape(n // 16, 16).T
    return np.ascontiguousarray(np.tile(w, (8, 1)).astype(np.int16))


def build_plan(feats, coords):
    order = np.argsort(coords[:, 0], kind="stable").astype(np.int32)
    pos = np.empty(N, np.int32)
    pos[order] = np.arange(N, dtype=np.int32)
    core_of = pos // N_LOC
    loc_dst = pos % N_LOC

    dsts, srcs = build_pairs(coords)

    per_core = [[None] * 125 for _ in range(N_CORES)]
    for t in TAPS:
        d, s = dsts[t], srcs[t]
        cd = core_of[d]
        for c in range(N_CORES):
            m = cd == c
            dl = loc_dst[d[m]]
            sg = s[m]
            o = np.argsort(dl, kind="stable")
            per_core[c][t] = (dl[o], sg[o])

    glob2loc = np.full((N_CORES, N), -1, np.int32)
    extras = []
    for c in range(N_CORES):
        dg = order[c * N_LOC : (c + 1) * N_LOC]
        glob2loc[c, dg] = np.arange(N_LOC, dtype=np.int32)
        need = np.unique(np.concatenate([per_core[c][t][1] for t in TAPS]))
        ex = need[glob2loc[c, need] < 0]
        glob2loc[c, ex] = N_LOC + np.arange(len(ex), dtype=np.int32)
        extras.append(ex)
    n_src = [N_LOC + len(e) for e in extras]
    SRC_ROWS = max(n_src)
    assert SRC_ROWS <= 32600, n_src
    feats_loc = np.zeros((N_CORES, SRC_ROWS, 128), ml_dtypes.bfloat16)
    feats_T = np.zeros((N_CORES, 128, NT), ml_dtypes.bfloat16)
    fb = feats.astype(ml_dtypes.bfloat16)
    for c in range(N_CORES):
        dg = order[c * N_LOC : (c + 1) * N_LOC]
        feats_loc[c, :N_LOC, :DIM] = fb[dg]
        feats_loc[c, N_LOC : n_src[c], :DIM] = fb[extras[c]]
        feats_T[c, :DIM, :N_LOC] = fb[dg].T

    # segments: one per tap, split by dst-halves while too wide for one op
    segments = []  # (tap, dst_lo, dst_hi, w)
    stack = [(t, 0, N_LOC) for t in TAPS]
    while stack:
        t, lo, hi = stack.pop(0)
        mx = 0
        for c in range(N_CORES):
            dl, _ = per_core[c][t]
            mx = max(mx, int(np.sum((dl >= lo) & (dl < hi))))
        w = max(1, (mx + 127) // 128)
        if w > 8:
            mid = (lo + hi) // 2
            stack = [(t, lo, mid), (t, mid, hi)] + stack
        else:
            segments.append((t, lo, hi, w))
    W = sum(s[3] for s in segments)

    gidx = np.zeros((N_CORES, W * 128), np.int32)
    sidx = np.full((N_CORES, W * 128), TRASH, np.int32)
    col = 0
    seg_cols = []
    for (t, lo, hi, w) in segments:
        seg_cols.append(col)
        for c in range(N_CORES):
            dl, sg = per_core[c][t]
            m = (dl >= lo) & (dl < hi)
            dls = dl[m]
            lids = glob2loc[c, sg[m]]
            n = len(dls)
            base = col * 128
            gidx[c, base : base + n] = lids
            sidx[c, base : base + n] = dls
        col += w
    assert col == W

    gidx_w = np.stack([wrap16(gidx[c]) for c in range(N_CORES)])
    sidx_w = np.stack([wrap16(sidx[c]) for c in range(N_CORES)])
    return feats_loc, feats_T, gidx_w, sidx_w, segments, seg_cols, W, order, SRC_ROWS


def _install_axon_profile_hook():
    try:
        import antenv

        if "antenv.axon_hooks" not in sys.modules:
            mod = types.ModuleType("antenv.axon_hooks")
            hook = [None]
            mod.set_axon_ntff_profile_hook = lambda h: hook.__setitem__(0, h)
            mod.get_axon_ntff_profile_hook = lambda: hook[0]
            sys.modules["antenv.axon_hooks"] = mod
            antenv.axon_hooks = mod
        from antenv.axon_hooks import (
            get_axon_ntff_profile_hook,
            set_axon_ntff_profile_hook,
        )

        if get_axon_ntff_profile_hook() is None:
            from trn_agent_boot.trn_boot import _ntff_profile_via_ctypes

            set_axon_ntff_profile_hook(
                _ntff_profile_via_ctypes("/opt/axon/libaxon_pjrt.so")
            )
    except Exception:
        pass


def build_program(segments, seg_cols, W, SRC_ROWS):
    import concourse.bacc as bacc
    import concourse.mybir as mybir
    import concourse.tile as tile
    from concourse.masks import make_identity

    nc = bacc.Bacc(
        "TRN2", num_devices=N_CORES, debug=False, target_bir_lowering=False,
        num_swdge_queues=4,
    )
    f32 = mybir.dt.float32
    bf16 = mybir.dt.bfloat16
    i16 = mybir.dt.int16

    feats_d = nc.dram_tensor("feats_loc", [SRC_ROWS, 128], bf16, kind="ExternalInput").ap()
    featsT_d = nc.dram_tensor("feats_T", [128, NT], bf16, kind="ExternalInput").ap()
    ktaps_d = nc.dram_tensor("ktaps", [128, 125 * 80], bf16, kind="ExternalInput").ap()
    gidx_d = nc.dram_tensor("gidx", [128, 8 * W], i16, kind="ExternalInput").ap()
    sidx_d = nc.dram_tensor("sidx", [128, 8 * W], i16, kind="ExternalInput").ap()
    out_d = nc.dram_tensor("out", [NT, 80], f32, kind="ExternalOutput").ap()
    tbl = [
        nc.dram_tensor(f"tbl{i}", [NTT, 128], bf16, kind="ExternalOutput").ap()
        for i in range(N_TBL)
    ]

    with tile.TileContext(nc) as tc:
        with (
            tc.tile_pool(name="const", bufs=1) as cpool,
            tc.tile_pool(name="gath", bufs=6) as gpool,
            tc.tile_pool(name="xsb", bufs=4) as xpool,
            tc.tile_pool(name="ysb", bufs=4) as ypool,
            tc.tile_pool(name="osb", bufs=3) as opool,
            tc.tile_pool(name="psX", bufs=3, space="PSUM") as psX,
            tc.tile_pool(name="psY", bufs=3, space="PSUM") as psY,
            tc.tile_pool(name="psB", bufs=2, space="PSUM") as psB,
        ):
            gsb = cpool.tile([128, 8 * W], i16)
            nc.sync.dma_start(out=gsb[:], in_=gidx_d[:])
            ssb = cpool.tile([128, 8 * W], i16)
            nc.sync.dma_start(out=ssb[:], in_=sidx_d[:])
            ident = cpool.tile([128, 128], bf16)
            make_identity(nc, ident[:])
            ksb = cpool.tile([128, 125 * 80], bf16)
            nc.sync.dma_start(out=ksb[:], in_=ktaps_d[:])
            ftsb = cpool.tile([128, NT], bf16)
            nc.sync.dma_start(out=ftsb[:], in_=featsT_d[:])

            cp_rr = [0]

            def do_copy(out, in_):
                cp_rr[0] += 1
                if cp_rr[0] % 2:
                    nc.scalar.copy(out=out, in_=in_)
                else:
                    nc.vector.tensor_copy(out=out, in_=in_)

            def emit_center(b):
                ps = psB.tile([128, 80], f32, tag="ops")
                d0 = b * 128
                nc.tensor.matmul(
                    out=ps[:],
                    lhsT=ftsb[:, d0 : d0 + 128],
                    rhs=ksb[:, CENTER_TAP * 80 : (CENTER_TAP + 1) * 80],
                    start=True,
                    stop=True,
                )
                ob = opool.tile([128, 80], f32, tag="ob")
                do_copy(ob[:], ps[:])
                nc.sync.dma_start(out=out_d[d0 : d0 + 128, :], in_=ob[:])

            def emit_segment(i):
                t, lo, hi, w = segments[i]
                c0 = seg_cols[i]
                ntok = w * 128
                q = i % 4
                gt = gpool.tile([128, w, 128], bf16, tag="gt")
                nc.gpsimd.dma_gather(
                    out_ap=gt[:],
                    in_ap=feats_d[:],
                    idxs_ap=gsb[:, c0 * 8 : c0 * 8 + 8 * w],
                    num_idxs=ntok,
                    num_idxs_reg=ntok,
                    elem_size=128,
                    queue_num=q,
                )
                ysb = ypool.tile([128, w, 80], bf16, tag="ysb")
                for k0 in range(0, w, SUB):
                    kn = min(SUB, w - k0)
                    xps = psX.tile([128, kn, 128], bf16, tag="xps")
                    for k in range(kn):
                        nc.tensor.transpose(
                            out=xps[:, k, :],
                            in_=gt[:, k0 + k, :],
                            identity=ident[:],
                        )
                    xsb = xpool.tile([128, kn, 128], bf16, tag="xsb")
                    do_copy(xsb[:], xps[:])
                    yps = psY.tile([128, kn, 80], f32, tag="yps")
                    for k in range(kn):
                        nc.tensor.matmul(
                            out=yps[:, k, :],
                            lhsT=xsb[:, k, :],
                            rhs=ksb[:, t * 80 : (t + 1) * 80],
                            start=True,
                            stop=True,
                        )
                    do_copy(ysb[:, k0 : k0 + kn, :], yps[:])
                nc.gpsimd.dma_scatter_add(
                    out_ap=tbl[q][:, :80],
                    in_ap=ysb[:],
                    idxs_ap=ssb[:, c0 * 8 : c0 * 8 + 8 * w],
                    num_idxs=ntok,
                    num_idxs_reg=ntok,
                    elem_size=80,
                    elem_step=128,
                    queue_num=q,
                )

            # interleave: sparse segments with center blocks spread between
            nseg = len(segments)
            cb = 0
            for i in range(nseg):
                emit_segment(i)
                want = (i + 1) * NBLK // nseg
                while cb < want:
                    emit_center(cb)
                    cb += 1
            while cb < NBLK:
                emit_center(cb)
                cb += 1

    print("tile build done", file=sys.stderr)
    nc.compile()
    print("bacc compile done", file=sys.stderr)
    return nc


_LAST = {"exec_time_ns": None, "results": None}


def kernel(feats, weight, w_sc0, w_sc1, coords):
    feats = np.ascontiguousarray(np.asarray(feats, np.float32))
    weight = np.asarray(weight, np.float32)
    w_sc0 = np.asarray(w_sc0, np.float32)
    w_sc1 = np.asarray(w_sc1, np.float32)
    coords = np.asarray(coords, np.int32)

    K = make_kernel_np(weight)
    K[CENTER_TAP] = K[CENTER_TAP] + w_sc_embed(w_sc0, w_sc1)
    ktaps = np.zeros((128, 125 * 80), np.float32)
    ktaps[:DIM] = K.transpose(1, 0, 2).reshape(DIM, 125 * 80)
    ktaps = np.ascontiguousarray(ktaps.astype(ml_dtypes.bfloat16))

    (
        feats_loc,
        feats_T,
        gidx_w,
        sidx_w,
        segments,
        seg_cols,
        W,
        order,
        SRC_ROWS,
    ) = build_plan(feats, coords)
    print(
        f"plan: W={W} nseg={len(segments)} SRC_ROWS={SRC_ROWS}",
        file=sys.stderr,
    )

    _install_axon_profile_hook()
    from concourse.bass_utils import run_bass_kernel_spmd

    nc = build_program(segments, seg_cols, W, SRC_ROWS)
    in_maps = [
        {
            "feats_loc": feats_loc[c],
            "feats_T": feats_T[c],
            "ktaps": ktaps,
            "gidx": gidx_w[c],
            "sidx": sidx_w[c],
        }
        for c in range(N_CORES)
    ]
    import os

    trace = os.environ.get("BASS_KERNEL_TRACE", "0") == "1"
    import time as _time

    res = None
    last_exc = None
    for attempt in range(4):
        try:
            res = run_bass_kernel_spmd(
                nc,
                in_maps,
                core_ids=list(range(N_CORES)),
                trace=trace and attempt == 0,
            )
            break
        except Exception as e:  # device flake: retry, later attempts untraced
            last_exc = e
            print(f"run attempt {attempt} failed: {e}", file=sys.stderr)
            _time.sleep(3.0)
    if res is None:
        raise last_exc
    print("hw run done", file=sys.stderr)
    _LAST["exec_time_ns"] = res.exec_time_ns
    _LAST["results"] = res
    out = np.empty((N, DIM), np.float32)
    for c in range(N_CORES):
        r = res.results[c]
        acc = np.asarray(r["out"])[:N_LOC, :DIM].astype(np.float32)
        for i in range(N_TBL):
            acc = acc + np.asarray(r[f"tbl{i}"])[:N_LOC, :DIM].astype(np.float32)
        out[order[c * N_LOC : (c + 1) * N_LOC]] = acc
    return out


# revision 4
# speedup vs baseline: 1.0409x; 1.0409x over previous
"""Trainium2 Bass kernel V3 for sparse 3D voxel convolution (e3nn-style, 5^3 taps).

Sharding: data-parallel over the N=200000 sparse voxels, sorted by x-plane and
split into 8 contiguous slabs of 25000 destination voxels; each core holds a
local bf16 feature table (slab + halo, <32k rows, int16 gather ids).

Single-phase per-tap pipeline per core:
  - center tap + residual: the slab's features live transposed in SBUF
    ([feat, dst] bf16 strip); one matmul per 128-dst block against the
    center kernel accumulates in PSUM and stores contiguous f32 output rows.
  - 124 sparse taps, one gather + one scatter-add per tap (pair lists padded
    to the max count over cores; pads gather row 0 and scatter into a trash
    row): bf16 dma_gather (256B rows) -> PE transpose (bf16 identity) ->
    matmul against the tap kernel -> bf16 dma_scatter_add (160B payload,
    256B row pitch) into one of 4 bf16 tables, table == queue so RMW stays
    ordered. Destinations are unique within a tap, so no updates are lost.

Host sums out + the 4 tables (the conv part is ~1% of the residual, bf16
accumulation is far inside the tolerance).
"""

import sys
import types

import numpy as np
import ml_dtypes

NRB = 8
RAD = 2.5
GRID = 192
N = 200000
DIM = 80
ALPHA = 1.0 / np.sqrt(48.0)
N_CORES = 8
N_LOC = N // N_CORES            # 25000 dst voxels per core
NBLK = (N_LOC + 127) // 128     # 196 out blocks
NT = NBLK * 128                 # 25088 out rows
TRASH = NT                      # scatter pad row (tables have NT+1... rows)
NTT = NT + 128                  # table rows incl trash
SUB = 4                         # columns per PSUM tile
N_TBL = 4                       # scatter tables == queues

_ax = np.arange(-2.0, 3.0, dtype=np.float32)
LATTICE = np.stack(np.meshgrid(_ax, _ax, _ax, indexing="ij"), -1)
PERM = np.arange(125).reshape(5, 5, 5).transpose(2, 1, 0).reshape(-1)
OFFS = LATTICE.reshape(-1, 3).astype(np.int32)[PERM]
CENTER_TAP = 62
TAPS = [t for t in range(125) if t != CENTER_TAP]


def _radial_emb():
    d = np.linalg.norm(LATTICE, axis=-1)
    centers = np.linspace(0.0, RAD, NRB + 2)[1:-1]
    step = centers[1] - centers[0]
    t = (d[..., None] - centers) / step
    inside = np.abs(t) < 1.0
    safe = np.where(inside, 1.0 - t * t, 1.0)
    return (1.14136 * np.exp(2.0) * np.where(inside, np.exp(-2.0 / safe), 0.0)).astype(
        np.float32
    )


EMB = _radial_emb().reshape(-1, NRB)


def _sph():
    n = np.linalg.norm(LATTICE, axis=-1, keepdims=True)
    u = np.where(n > 0, LATTICE / np.maximum(n, 1e-9), 0.0)
    return np.concatenate([np.ones_like(n), np.sqrt(3.0) * u], -1).astype(np.float32)


SH = _sph().reshape(-1, 4)


def make_kernel_np(weight):
    w = (EMB @ weight.astype(np.float32)) / 125.0
    w1 = w[:, :1024].reshape(125, 32, 32)
    w2 = w[:, 1024:1536].reshape(125, 32, 16)
    w3 = w[:, 1536:1792].reshape(125, 16, 16)
    w4 = w[:, 1792:].reshape(125, 16, 32)
    s0 = SH[:, 0]
    v = SH[:, 1:]
    eye3 = np.eye(3, dtype=w.dtype)
    K00 = ALPHA * w1 * s0[:, None, None]
    K01 = ALPHA * np.einsum("pik,pm->pikm", w2, v).reshape(125, 32, 48)
    K11 = ALPHA * np.einsum(
        "pik,mn->pimkn", w3 * s0[:, None, None], eye3
    ).reshape(125, 48, 48)
    K10 = (ALPHA / np.sqrt(3.0)) * np.einsum("pik,pm->pimk", w4, v).reshape(125, 48, 32)
    K = np.concatenate(
        [np.concatenate([K00, K01], 2), np.concatenate([K10, K11], 2)], 1
    )
    return K[PERM]


def w_sc_embed(w_sc0, w_sc1):
    W = np.zeros((80, 80), np.float32)
    W[:32, :32] = w_sc0 / np.sqrt(32.0)
    blk = np.zeros((48, 48), np.float32)
    for m in range(3):
        blk[m::3, m::3] = w_sc1 / np.sqrt(16.0)
    W[32:, 32:] = blk
    return W


def build_pairs(coords):
    idx_vol = np.full(GRID * GRID * GRID, -1, np.int32)
    lin = (coords[:, 0].astype(np.int64) * GRID + coords[:, 1]) * GRID + coords[:, 2]
    idx_vol[lin] = np.arange(N, dtype=np.int32)
    all_i = np.arange(N, dtype=np.int32)
    dsts, srcs = [], []
    for t in range(125):
        if t == CENTER_TAP:
            dsts.append(None)
            srcs.append(None)
            continue
        c = coords + OFFS[t]
        ok = np.all((c >= 0) & (c < GRID), axis=1)
        cl = (c[:, 0].astype(np.int64) * GRID + c[:, 1]) * GRID + c[:, 2]
        cl = np.clip(cl, 0, GRID**3 - 1)
        nb = idx_vol[cl]
        valid = ok & (nb >= 0)
        dsts.append(all_i[valid])
        srcs.append(nb[valid])
    return dsts, srcs


def wrap16(a):
    """Token stream [n] -> [128, n//16] int16 (16-partition wrap, 8x replicated)."""
    n = a.shape[0]
    w = a.reshape(n // 16, 16).T
    return np.ascontiguousarray(np.tile(w, (8, 1)).astype(np.int16))


def build_plan(feats, coords):
    order = np.argsort(coords[:, 0], kind="stable").astype(np.int32)
    pos = np.empty(N, np.int32)
    pos[order] = np.arange(N, dtype=np.int32)
    core_of = pos // N_LOC
    loc_dst = pos % N_LOC

    dsts, srcs = build_pairs(coords)

    per_core = [[None] * 125 for _ in range(N_CORES)]
    for t in TAPS:
        d, s = dsts[t], srcs[t]
        cd = core_of[d]
        for c in range(N_CORES):
            m = cd == c
            dl = loc_dst[d[m]]
            sg = s[m]
            o = np.argsort(dl, kind="stable")
            per_core[c][t] = (dl[o], sg[o])

    glob2loc = np.full((N_CORES, N), -1, np.int32)
    extras = []
    for c in range(N_CORES):
        dg = order[c * N_LOC : (c + 1) * N_LOC]
        glob2loc[c, dg] = np.arange(N_LOC, dtype=np.int32)
        need = np.unique(np.concatenate([per_core[c][t][1] for t in TAPS]))
        ex = need[glob2loc[c, need] < 0]
        glob2loc[c, ex] = N_LOC + np.arange(len(ex), dtype=np.int32)
        extras.append(ex)
    n_src = [N_LOC + len(e) for e in extras]
    SRC_ROWS = max(n_src)
    assert SRC_ROWS <= 32600, n_src
    feats_loc = np.zeros((N_CORES, SRC_ROWS, 128), ml_dtypes.bfloat16)
    feats_T = np.zeros((N_CORES, 128, NT), ml_dtypes.bfloat16)
    fb = feats.astype(ml_dtypes.bfloat16)
    for c in range(N_CORES):
        dg = order[c * N_LOC : (c + 1) * N_LOC]
        feats_loc[c, :N_LOC, :DIM] = fb[dg]
        feats_loc[c, N_LOC : n_src[c], :DIM] = fb[extras[c]]
        feats_T[c, :DIM, :N_LOC] = fb[dg].T

    # segments: one per tap, split by dst-halves while too wide for one op
    segments = []  # (tap, dst_lo, dst_hi, w)
    stack = [(t, 0, N_LOC) for t in TAPS]
    while stack:
        t, lo, hi = stack.pop(0)
        mx = 0
        for c in range(N_CORES):
            dl, _ = per_core[c][t]
            mx = max(mx, int(np.sum((dl >= lo) & (dl < hi))))
        w = max(1, (mx + 127) // 128)
        if w > 8:
            mid = (lo + hi) // 2
            stack = [(t, lo, mid), (t, mid, hi)] + stack
        else:
            segments.append((t, lo, hi, w))
    W = sum(s[3] for s in segments)

    gidx = np.zeros((N_CORES, W * 128), np.int32)
    sidx = np.full((N_CORES, W * 128), TRASH, np.int32)
    col = 0
    seg_cols = []
    for (t, lo, hi, w) in segments:
        seg_cols.append(col)
        for c in range(N_CORES):
            dl, sg = per_core[c][t]
            m = (dl >= lo) & (dl < hi)
            dls = dl[m]
            lids = glob2loc[c, sg[m]]
            n = len(dls)
            base = col * 128
            gidx[c, base : base + n] = lids
            sidx[c, base : base + n] = dls
        col += w
    assert col == W

    gidx_w = np.stack([wrap16(gidx[c]) for c in range(N_CORES)])
    sidx_w = np.stack([wrap16(sidx[c]) for c in range(N_CORES)])
    return feats_loc, feats_T, gidx_w, sidx_w, segments, seg_cols, W, order, SRC_ROWS


def _install_axon_profile_hook():
    try:
        import antenv

        if "antenv.axon_hooks" not in sys.modules:
            mod = types.ModuleType("antenv.axon_hooks")
            hook = [None]
            mod.set_axon_ntff_profile_hook = lambda h: hook.__setitem__(0, h)
            mod.get_axon_ntff_profile_hook = lambda: hook[0]
            sys.modules["antenv.axon_hooks"] = mod
            antenv.axon_hooks = mod
        from antenv.axon_hooks import (
            get_axon_ntff_profile_hook,
            set_axon_ntff_profile_hook,
        )

        if get_axon_ntff_profile_hook() is None:
            from trn_agent_boot.trn_boot import _ntff_profile_via_ctypes

            set_axon_ntff_profile_hook(
                _ntff_profile_via_ctypes("/opt/axon/libaxon_pjrt.so")
            )
    except Exception:
        pass


def build_program(segments, seg_cols, W, SRC_ROWS):
    import concourse.bacc as bacc
    import concourse.mybir as mybir
    import concourse.tile as tile
    from concourse.masks import make_identity

    nc = bacc.Bacc(
        "TRN2", num_devices=N_CORES, debug=False, target_bir_lowering=False,
        num_swdge_queues=4,
    )
    f32 = mybir.dt.float32
    bf16 = mybir.dt.bfloat16
    i16 = mybir.dt.int16

    feats_d = nc.dram_tensor("feats_loc", [SRC_ROWS, 128], bf16, kind="ExternalInput").ap()
    featsT_d = nc.dram_tensor("feats_T", [128, NT], bf16, kind="ExternalInput").ap()
    ktaps_d = nc.dram_tensor("ktaps", [128, 125 * 80], bf16, kind="ExternalInput").ap()
    gidx_d = nc.dram_tensor("gidx", [128, 8 * W], i16, kind="ExternalInput").ap()
    sidx_d = nc.dram_tensor("sidx", [128, 8 * W], i16, kind="ExternalInput").ap()
    out_d = nc.dram_tensor("out", [NT, 80], f32, kind="ExternalOutput").ap()
    tbl = [
        nc.dram_tensor(f"tbl{i}", [NTT, 128], bf16, kind="ExternalOutput").ap()
        for i in range(N_TBL)
    ]

    with tile.TileContext(nc) as tc:
        with (
            tc.tile_pool(name="const", bufs=1) as cpool,
            tc.tile_pool(name="gath", bufs=6) as gpool,
            tc.tile_pool(name="xsb", bufs=4) as xpool,
            tc.tile_pool(name="ysb", bufs=4) as ypool,
            tc.tile_pool(name="osb", bufs=3) as opool,
            tc.tile_pool(name="psX", bufs=3, space="PSUM") as psX,
            tc.tile_pool(name="psY", bufs=3, space="PSUM") as psY,
            tc.tile_pool(name="psB", bufs=2, space="PSUM") as psB,
        ):
            gsb = cpool.tile([128, 8 * W], i16)
            nc.sync.dma_start(out=gsb[:], in_=gidx_d[:])
            ident = cpool.tile([128, 128], bf16)
            make_identity(nc, ident[:])
            ksb = cpool.tile([128, 125 * 80], bf16)
            nc.sync.dma_start(out=ksb[:], in_=ktaps_d[:])
            ssb = cpool.tile([128, 8 * W], i16)
            nc.sync.dma_start(out=ssb[:], in_=sidx_d[:])
            ftsb = cpool.tile([128, NT], bf16)
            nc.sync.dma_start(out=ftsb[:], in_=featsT_d[:])

            cp_rr = [0]

            def do_copy(out, in_):
                cp_rr[0] += 1
                if cp_rr[0] % 2:
                    nc.scalar.copy(out=out, in_=in_)
                else:
                    nc.vector.tensor_copy(out=out, in_=in_)

            def emit_center(b):
                ps = psB.tile([128, 80], f32, tag="ops")
                d0 = b * 128
                nc.tensor.matmul(
                    out=ps[:],
                    lhsT=ftsb[:, d0 : d0 + 128],
                    rhs=ksb[:, CENTER_TAP * 80 : (CENTER_TAP + 1) * 80],
                    start=True,
                    stop=True,
                )
                ob = opool.tile([128, 80], f32, tag="ob")
                do_copy(ob[:], ps[:])
                nc.sync.dma_start(out=out_d[d0 : d0 + 128, :], in_=ob[:])

            def emit_segment(i):
                t, lo, hi, w = segments[i]
                c0 = seg_cols[i]
                ntok = w * 128
                q = i % 4
                gt = gpool.tile([128, w, 128], bf16, tag="gt")
                nc.gpsimd.dma_gather(
                    out_ap=gt[:],
                    in_ap=feats_d[:],
                    idxs_ap=gsb[:, c0 * 8 : c0 * 8 + 8 * w],
                    num_idxs=ntok,
                    num_idxs_reg=ntok,
                    elem_size=128,
                    queue_num=q,
                )
                ysb = ypool.tile([128, w, 80], bf16, tag="ysb")
                for k0 in range(0, w, SUB):
                    kn = min(SUB, w - k0)
                    xps = psX.tile([128, kn, 128], bf16, tag="xps")
                    for k in range(kn):
                        nc.tensor.transpose(
                            out=xps[:, k, :],
                            in_=gt[:, k0 + k, :],
                            identity=ident[:],
                        )
                    xsb = xpool.tile([128, kn, 128], bf16, tag="xsb")
                    do_copy(xsb[:], xps[:])
                    yps = psY.tile([128, kn, 80], f32, tag="yps")
                    for k in range(kn):
                        nc.tensor.matmul(
                            out=yps[:, k, :],
                            lhsT=xsb[:, k, :],
                            rhs=ksb[:, t * 80 : (t + 1) * 80],
                            start=True,
                            stop=True,
                        )
                    do_copy(ysb[:, k0 : k0 + kn, :], yps[:])
                nc.gpsimd.dma_scatter_add(
                    out_ap=tbl[q][:, :80],
                    in_ap=ysb[:],
                    idxs_ap=ssb[:, c0 * 8 : c0 * 8 + 8 * w],
                    num_idxs=ntok,
                    num_idxs_reg=ntok,
                    elem_size=80,
                    elem_step=128,
                    queue_num=q,
                )

            # interleave: sparse segments with center blocks spread between
            # front-load center blocks into segments [16, 72): by then the
            # feature strip has loaded, and they finish long before the
            # gather/scatter stream does.
            nseg = len(segments)
            cb = 0
            for i in range(nseg):
                emit_segment(i)
                want = 0 if i < 16 else min(NBLK, (i - 15) * NBLK // 56)
                while cb < want:
                    emit_center(cb)
                    cb += 1
            while cb < NBLK:
                emit_center(cb)
                cb += 1

    print("tile build done", file=sys.stderr)
    nc.compile()
    print("bacc compile done", file=sys.stderr)
    return nc


_LAST = {"exec_time_ns": None, "results": None}


def kernel(feats, weight, w_sc0, w_sc1, coords):
    feats = np.ascontiguousarray(np.asarray(feats, np.float32))
    weight = np.asarray(weight, np.float32)
    w_sc0 = np.asarray(w_sc0, np.float32)
    w_sc1 = np.asarray(w_sc1, np.float32)
    coords = np.asarray(coords, np.int32)

    K = make_kernel_np(weight)
    K[CENTER_TAP] = K[CENTER_TAP] + w_sc_embed(w_sc0, w_sc1)
    ktaps = np.zeros((128, 125 * 80), np.float32)
    ktaps[:DIM] = K.transpose(1, 0, 2).reshape(DIM, 125 * 80)
    ktaps = np.ascontiguousarray(ktaps.astype(ml_dtypes.bfloat16))

    (
        feats_loc,
        feats_T,
        gidx_w,
        sidx_w,
        segments,
        seg_cols,
        W,
        order,
        SRC_ROWS,
    ) = build_plan(feats, coords)
    print(
        f"plan: W={W} nseg={len(segments)} SRC_ROWS={SRC_ROWS}",
        file=sys.stderr,
    )

    _install_axon_profile_hook()
    from concourse.bass_utils import run_bass_kernel_spmd

    nc = build_program(segments, seg_cols, W, SRC_ROWS)
    in_maps = [
        {
            "feats_loc": feats_loc[c],
            "feats_T": feats_T[c],
            "ktaps": ktaps,
            "gidx": gidx_w[c],
            "sidx": sidx_w[c],
        }
        for c in range(N_CORES)
    ]
    import os

    trace = os.environ.get("BASS_KERNEL_TRACE", "0") == "1"
    import time as _time

    res = None
    last_exc = None
    for attempt in range(4):
        try:
            res = run_bass_kernel_spmd(
                nc,
                in_maps,
                core_ids=list(range(N_CORES)),
                trace=trace and attempt == 0,
            )
            break
        except Exception as e:  # device flake: retry, later attempts untraced
            last_exc = e
            print(f"run attempt {attempt} failed: {e}", file=sys.stderr)
            _time.sleep(3.0)
    if res is None:
        raise last_exc
    print("hw run done", file=sys.stderr)
    _LAST["exec_time_ns"] = res.exec_time_ns
    _LAST["results"] = res
    out = np.empty((N, DIM), np.float32)
    for c in range(N_CORES):
        r = res.results[c]
        acc = np.asarray(r["out"])[:N_LOC, :DIM].astype(np.float32)
        for i in range(N_TBL):
            acc = acc + np.asarray(r[f"tbl{i}"])[:N_LOC, :DIM].astype(np.float32)
        out[order[c * N_LOC : (c + 1) * N_LOC]] = acc
    return out


# revision 5
# speedup vs baseline: 1.1664x; 1.1205x over previous
"""Trainium2 Bass kernel V3 for sparse 3D voxel convolution (e3nn-style, 5^3 taps).

Sharding: data-parallel over the N=200000 sparse voxels, sorted by x-plane and
split into 8 contiguous slabs of 25000 destination voxels; each core holds a
local bf16 feature table (slab + halo, <32k rows, int16 gather ids).

Single-phase per-tap pipeline per core:
  - center tap + residual: the slab's features live transposed in SBUF
    ([feat, dst] bf16 strip); one matmul per 128-dst block against the
    center kernel accumulates in PSUM and stores contiguous f32 output rows.
  - 124 sparse taps, one gather + one scatter-add per tap (pair lists padded
    to the max count over cores; pads gather row 0 and scatter into a trash
    row): bf16 dma_gather (256B rows) -> PE transpose (bf16 identity) ->
    matmul against the tap kernel -> bf16 dma_scatter_add (160B payload,
    256B row pitch) into one of 4 bf16 tables, table == queue so RMW stays
    ordered. Destinations are unique within a tap, so no updates are lost.

Host sums out + the 4 tables (the conv part is ~1% of the residual, bf16
accumulation is far inside the tolerance).
"""

import sys
import types

import numpy as np
import ml_dtypes

NRB = 8
RAD = 2.5
GRID = 192
N = 200000
DIM = 80
ALPHA = 1.0 / np.sqrt(48.0)
N_CORES = 8
N_LOC = N // N_CORES            # 25000 dst voxels per core
NBLK = (N_LOC + 127) // 128     # 196 out blocks
NT = NBLK * 128                 # 25088 out rows
TRASH = NT                      # scatter pad row (tables have NT+1... rows)
NTT = NT + 128                  # table rows incl trash
SUB = 4                         # columns per PSUM tile
N_TBL = 4                       # scatter tables == queues

_ax = np.arange(-2.0, 3.0, dtype=np.float32)
LATTICE = np.stack(np.meshgrid(_ax, _ax, _ax, indexing="ij"), -1)
PERM = np.arange(125).reshape(5, 5, 5).transpose(2, 1, 0).reshape(-1)
OFFS = LATTICE.reshape(-1, 3).astype(np.int32)[PERM]
CENTER_TAP = 62
TAPS = [t for t in range(125) if t != CENTER_TAP]


def _radial_emb():
    d = np.linalg.norm(LATTICE, axis=-1)
    centers = np.linspace(0.0, RAD, NRB + 2)[1:-1]
    step = centers[1] - centers[0]
    t = (d[..., None] - centers) / step
    inside = np.abs(t) < 1.0
    safe = np.where(inside, 1.0 - t * t, 1.0)
    return (1.14136 * np.exp(2.0) * np.where(inside, np.exp(-2.0 / safe), 0.0)).astype(
        np.float32
    )


EMB = _radial_emb().reshape(-1, NRB)


def _sph():
    n = np.linalg.norm(LATTICE, axis=-1, keepdims=True)
    u = np.where(n > 0, LATTICE / np.maximum(n, 1e-9), 0.0)
    return np.concatenate([np.ones_like(n), np.sqrt(3.0) * u], -1).astype(np.float32)


SH = _sph().reshape(-1, 4)


def make_kernel_np(weight):
    w = (EMB @ weight.astype(np.float32)) / 125.0
    w1 = w[:, :1024].reshape(125, 32, 32)
    w2 = w[:, 1024:1536].reshape(125, 32, 16)
    w3 = w[:, 1536:1792].reshape(125, 16, 16)
    w4 = w[:, 1792:].reshape(125, 16, 32)
    s0 = SH[:, 0]
    v = SH[:, 1:]
    eye3 = np.eye(3, dtype=w.dtype)
    K00 = ALPHA * w1 * s0[:, None, None]
    K01 = ALPHA * np.einsum("pik,pm->pikm", w2, v).reshape(125, 32, 48)
    K11 = ALPHA * np.einsum(
        "pik,mn->pimkn", w3 * s0[:, None, None], eye3
    ).reshape(125, 48, 48)
    K10 = (ALPHA / np.sqrt(3.0)) * np.einsum("pik,pm->pimk", w4, v).reshape(125, 48, 32)
    K = np.concatenate(
        [np.concatenate([K00, K01], 2), np.concatenate([K10, K11], 2)], 1
    )
    return K[PERM]


def w_sc_embed(w_sc0, w_sc1):
    W = np.zeros((80, 80), np.float32)
    W[:32, :32] = w_sc0 / np.sqrt(32.0)
    blk = np.zeros((48, 48), np.float32)
    for m in range(3):
        blk[m::3, m::3] = w_sc1 / np.sqrt(16.0)
    W[32:, 32:] = blk
    return W


def build_pairs(coords):
    idx_vol = np.full(GRID * GRID * GRID, -1, np.int32)
    lin = (coords[:, 0].astype(np.int64) * GRID + coords[:, 1]) * GRID + coords[:, 2]
    idx_vol[lin] = np.arange(N, dtype=np.int32)
    all_i = np.arange(N, dtype=np.int32)
    dsts, srcs = [], []
    for t in range(125):
        if t == CENTER_TAP:
            dsts.append(None)
            srcs.append(None)
            continue
        c = coords + OFFS[t]
        ok = np.all((c >= 0) & (c < GRID), axis=1)
        cl = (c[:, 0].astype(np.int64) * GRID + c[:, 1]) * GRID + c[:, 2]
        cl = np.clip(cl, 0, GRID**3 - 1)
        nb = idx_vol[cl]
        valid = ok & (nb >= 0)
        dsts.append(all_i[valid])
        srcs.append(nb[valid])
    return dsts, srcs


def wrap16(a):
    """Token stream [n] -> [128, n//16] int16 (16-partition wrap, 8x replicated)."""
    n = a.shape[0]
    w = a.reshape(n // 16, 16).T
    return np.ascontiguousarray(np.tile(w, (8, 1)).astype(np.int16))


def build_plan(feats, coords):
    order = np.argsort(coords[:, 0], kind="stable").astype(np.int32)
    pos = np.empty(N, np.int32)
    pos[order] = np.arange(N, dtype=np.int32)
    core_of = pos // N_LOC
    loc_dst = pos % N_LOC

    dsts, srcs = build_pairs(coords)

    per_core = [[None] * 125 for _ in range(N_CORES)]
    for t in TAPS:
        d, s = dsts[t], srcs[t]
        cd = core_of[d]
        for c in range(N_CORES):
            m = cd == c
            dl = loc_dst[d[m]]
            sg = s[m]
            o = np.argsort(dl, kind="stable")
            per_core[c][t] = (dl[o], sg[o])

    glob2loc = np.full((N_CORES, N), -1, np.int32)
    extras = []
    for c in range(N_CORES):
        dg = order[c * N_LOC : (c + 1) * N_LOC]
        glob2loc[c, dg] = np.arange(N_LOC, dtype=np.int32)
        need = np.unique(np.concatenate([per_core[c][t][1] for t in TAPS]))
        ex = need[glob2loc[c, need] < 0]
        glob2loc[c, ex] = N_LOC + np.arange(len(ex), dtype=np.int32)
        extras.append(ex)
    n_src = [N_LOC + len(e) for e in extras]
    SRC_ROWS = max(n_src)
    assert SRC_ROWS <= 32600, n_src
    feats_loc = np.zeros((N_CORES, SRC_ROWS, 128), ml_dtypes.bfloat16)
    feats_T = np.zeros((N_CORES, 128, NT), ml_dtypes.bfloat16)
    fb = feats.astype(ml_dtypes.bfloat16)
    for c in range(N_CORES):
        dg = order[c * N_LOC : (c + 1) * N_LOC]
        feats_loc[c, :N_LOC, :DIM] = fb[dg]
        feats_loc[c, N_LOC : n_src[c], :DIM] = fb[extras[c]]
        feats_T[c, :DIM, :N_LOC] = fb[dg].T

    # segments: one per tap, split by dst-halves while too wide for one op
    segments = []  # (tap, dst_lo, dst_hi, w)
    stack = [(t, 0, N_LOC) for t in TAPS]
    while stack:
        t, lo, hi = stack.pop(0)
        mx = 0
        for c in range(N_CORES):
            dl, _ = per_core[c][t]
            mx = max(mx, int(np.sum((dl >= lo) & (dl < hi))))
        w = max(1, (mx + 127) // 128)
        if w > 8:
            mid = (lo + hi) // 2
            stack = [(t, lo, mid), (t, mid, hi)] + stack
        else:
            segments.append((t, lo, hi, w, max(1, mx)))
    W = sum(s[3] for s in segments)

    gidx = np.zeros((N_CORES, W * 128), np.int32)
    sidx = np.full((N_CORES, W * 128), TRASH, np.int32)
    col = 0
    seg_cols = []
    for (t, lo, hi, w, mx) in segments:
        seg_cols.append(col)
        for c in range(N_CORES):
            dl, sg = per_core[c][t]
            m = (dl >= lo) & (dl < hi)
            dls = dl[m]
            lids = glob2loc[c, sg[m]]
            n = len(dls)
            base = col * 128
            gidx[c, base : base + n] = lids
            sidx[c, base : base + n] = dls
        col += w
    assert col == W

    gidx_w = np.stack([wrap16(gidx[c]) for c in range(N_CORES)])
    sidx_w = np.stack([wrap16(sidx[c]) for c in range(N_CORES)])
    return feats_loc, feats_T, gidx_w, sidx_w, segments, seg_cols, W, order, SRC_ROWS


def _install_axon_profile_hook():
    try:
        import antenv

        if "antenv.axon_hooks" not in sys.modules:
            mod = types.ModuleType("antenv.axon_hooks")
            hook = [None]
            mod.set_axon_ntff_profile_hook = lambda h: hook.__setitem__(0, h)
            mod.get_axon_ntff_profile_hook = lambda: hook[0]
            sys.modules["antenv.axon_hooks"] = mod
            antenv.axon_hooks = mod
        from antenv.axon_hooks import (
            get_axon_ntff_profile_hook,
            set_axon_ntff_profile_hook,
        )

        if get_axon_ntff_profile_hook() is None:
            from trn_agent_boot.trn_boot import _ntff_profile_via_ctypes

            set_axon_ntff_profile_hook(
                _ntff_profile_via_ctypes("/opt/axon/libaxon_pjrt.so")
            )
    except Exception:
        pass


ESPLIT_COLS = [0]


def build_program(segments, seg_cols, W, SRC_ROWS):
    ESPLIT_COLS[0] = seg_cols[16] if len(seg_cols) > 16 else W
    import concourse.bacc as bacc
    import concourse.mybir as mybir
    import concourse.tile as tile
    from concourse.masks import make_identity

    ESPLIT = ESPLIT_COLS[0]
    nc = bacc.Bacc(
        "TRN2", num_devices=N_CORES, debug=False, target_bir_lowering=False,
        num_swdge_queues=4,
    )
    f32 = mybir.dt.float32
    bf16 = mybir.dt.bfloat16
    i16 = mybir.dt.int16

    feats_d = nc.dram_tensor("feats_loc", [SRC_ROWS, 128], bf16, kind="ExternalInput").ap()
    featsT_d = nc.dram_tensor("feats_T", [128, NT], bf16, kind="ExternalInput").ap()
    ktaps_d = nc.dram_tensor("ktaps", [128, 125 * 80], bf16, kind="ExternalInput").ap()
    gidx0_d = nc.dram_tensor("gidx0", [128, 8 * ESPLIT], i16, kind="ExternalInput").ap()
    sidx0_d = nc.dram_tensor("sidx0", [128, 8 * ESPLIT], i16, kind="ExternalInput").ap()
    gidx_d = nc.dram_tensor("gidx", [128, 8 * (W - ESPLIT)], i16, kind="ExternalInput").ap()
    sidx_d = nc.dram_tensor("sidx", [128, 8 * (W - ESPLIT)], i16, kind="ExternalInput").ap()
    out_d = nc.dram_tensor("out", [NT, 80], f32, kind="ExternalOutput").ap()
    tbl = [
        nc.dram_tensor(f"tbl{i}", [NTT, 128], bf16, kind="ExternalOutput").ap()
        for i in range(N_TBL)
    ]

    with tile.TileContext(nc) as tc:
        with (
            tc.tile_pool(name="const", bufs=1) as cpool,
            tc.tile_pool(name="gath", bufs=6) as gpool,
            tc.tile_pool(name="xsb", bufs=4) as xpool,
            tc.tile_pool(name="ysb", bufs=4) as ypool,
            tc.tile_pool(name="osb", bufs=3) as opool,
            tc.tile_pool(name="psX", bufs=3, space="PSUM") as psX,
            tc.tile_pool(name="psY", bufs=3, space="PSUM") as psY,
            tc.tile_pool(name="psB", bufs=2, space="PSUM") as psB,
        ):
            gsb0 = cpool.tile([128, 8 * ESPLIT], i16)
            nc.sync.dma_start(out=gsb0[:], in_=gidx0_d[:])
            ssb0 = cpool.tile([128, 8 * ESPLIT], i16)
            nc.sync.dma_start(out=ssb0[:], in_=sidx0_d[:])
            ident = cpool.tile([128, 128], bf16)
            make_identity(nc, ident[:])
            ksb = cpool.tile([128, 125 * 80], bf16)
            nc.sync.dma_start(out=ksb[:], in_=ktaps_d[:])
            gsb = cpool.tile([128, 8 * (W - ESPLIT)], i16)
            nc.sync.dma_start(out=gsb[:], in_=gidx_d[:])
            ssb = cpool.tile([128, 8 * (W - ESPLIT)], i16)
            nc.sync.dma_start(out=ssb[:], in_=sidx_d[:])
            ftsb = cpool.tile([128, NT], bf16)
            nc.sync.dma_start(out=ftsb[:], in_=featsT_d[:])

            cp_rr = [0]

            def do_copy(out, in_):
                cp_rr[0] += 1
                if cp_rr[0] % 2:
                    nc.scalar.copy(out=out, in_=in_)
                else:
                    nc.vector.tensor_copy(out=out, in_=in_)

            def emit_center(b):
                ps = psB.tile([128, 80], f32, tag="ops")
                d0 = b * 128
                nc.tensor.matmul(
                    out=ps[:],
                    lhsT=ftsb[:, d0 : d0 + 128],
                    rhs=ksb[:, CENTER_TAP * 80 : (CENTER_TAP + 1) * 80],
                    start=True,
                    stop=True,
                )
                ob = opool.tile([128, 80], f32, tag="ob")
                do_copy(ob[:], ps[:])
                nc.sync.dma_start(out=out_d[d0 : d0 + 128, :], in_=ob[:])

            def emit_segment(i):
                t, lo, hi, w, mx = segments[i]
                c0 = seg_cols[i]
                q = i % 4
                if c0 < ESPLIT:
                    gslice = gsb0[:, c0 * 8 : c0 * 8 + 8 * w]
                    sslice = ssb0[:, c0 * 8 : c0 * 8 + 8 * w]
                else:
                    gslice = gsb[:, (c0 - ESPLIT) * 8 : (c0 - ESPLIT) * 8 + 8 * w]
                    sslice = ssb[:, (c0 - ESPLIT) * 8 : (c0 - ESPLIT) * 8 + 8 * w]
                gt = gpool.tile([128, w, 128], bf16, tag="gt")
                nc.gpsimd.dma_gather(
                    out_ap=gt[:],
                    in_ap=feats_d[:],
                    idxs_ap=gslice,
                    num_idxs=mx,
                    num_idxs_reg=mx,
                    elem_size=128,
                    queue_num=q,
                )
                ysb = ypool.tile([128, w, 80], bf16, tag="ysb")
                for k0 in range(0, w, SUB):
                    kn = min(SUB, w - k0)
                    xps = psX.tile([128, kn, 128], bf16, tag="xps")
                    for k in range(kn):
                        nc.tensor.transpose(
                            out=xps[:, k, :],
                            in_=gt[:, k0 + k, :],
                            identity=ident[:],
                        )
                    xsb = xpool.tile([128, kn, 128], bf16, tag="xsb")
                    do_copy(xsb[:], xps[:])
                    yps = psY.tile([128, kn, 80], f32, tag="yps")
                    for k in range(kn):
                        nc.tensor.matmul(
                            out=yps[:, k, :],
                            lhsT=xsb[:, k, :],
                            rhs=ksb[:, t * 80 : (t + 1) * 80],
                            start=True,
                            stop=True,
                        )
                    do_copy(ysb[:, k0 : k0 + kn, :], yps[:])
                nc.gpsimd.dma_scatter_add(
                    out_ap=tbl[q][:, :80],
                    in_ap=ysb[:],
                    idxs_ap=sslice,
                    num_idxs=mx,
                    num_idxs_reg=mx,
                    elem_size=80,
                    elem_step=128,
                    queue_num=q,
                )

            # interleave: sparse segments with center blocks spread between
            # front-load center blocks into segments [16, 72): by then the
            # feature strip has loaded, and they finish long before the
            # gather/scatter stream does.
            nseg = len(segments)
            cb = 0
            for i in range(nseg):
                emit_segment(i)
                want = 0 if i < 16 else min(NBLK, (i - 15) * NBLK // 56)
                while cb < want:
                    emit_center(cb)
                    cb += 1
            while cb < NBLK:
                emit_center(cb)
                cb += 1

    print("tile build done", file=sys.stderr)
    nc.compile()
    print("bacc compile done", file=sys.stderr)
    return nc


_LAST = {"exec_time_ns": None, "results": None}


def kernel(feats, weight, w_sc0, w_sc1, coords):
    feats = np.ascontiguousarray(np.asarray(feats, np.float32))
    weight = np.asarray(weight, np.float32)
    w_sc0 = np.asarray(w_sc0, np.float32)
    w_sc1 = np.asarray(w_sc1, np.float32)
    coords = np.asarray(coords, np.int32)

    K = make_kernel_np(weight)
    K[CENTER_TAP] = K[CENTER_TAP] + w_sc_embed(w_sc0, w_sc1)
    ktaps = np.zeros((128, 125 * 80), np.float32)
    ktaps[:DIM] = K.transpose(1, 0, 2).reshape(DIM, 125 * 80)
    ktaps = np.ascontiguousarray(ktaps.astype(ml_dtypes.bfloat16))

    (
        feats_loc,
        feats_T,
        gidx_w,
        sidx_w,
        segments,
        seg_cols,
        W,
        order,
        SRC_ROWS,
    ) = build_plan(feats, coords)
    print(
        f"plan: W={W} nseg={len(segments)} SRC_ROWS={SRC_ROWS}",
        file=sys.stderr,
    )

    _install_axon_profile_hook()
    from concourse.bass_utils import run_bass_kernel_spmd

    nc = build_program(segments, seg_cols, W, SRC_ROWS)
    es = ESPLIT_COLS[0]
    in_maps = [
        {
            "feats_loc": feats_loc[c],
            "feats_T": feats_T[c],
            "ktaps": ktaps,
            "gidx0": np.ascontiguousarray(gidx_w[c][:, : 8 * es]),
            "sidx0": np.ascontiguousarray(sidx_w[c][:, : 8 * es]),
            "gidx": np.ascontiguousarray(gidx_w[c][:, 8 * es :]),
            "sidx": np.ascontiguousarray(sidx_w[c][:, 8 * es :]),
        }
        for c in range(N_CORES)
    ]
    import os

    trace = os.environ.get("BASS_KERNEL_TRACE", "0") == "1"
    import time as _time

    res = None
    last_exc = None
    for attempt in range(4):
        try:
            res = run_bass_kernel_spmd(
                nc,
                in_maps,
                core_ids=list(range(N_CORES)),
                trace=trace and attempt == 0,
            )
            break
        except Exception as e:  # device flake: retry, later attempts untraced
            last_exc = e
            print(f"run attempt {attempt} failed: {e}", file=sys.stderr)
            _time.sleep(3.0)
    if res is None:
        raise last_exc
    print("hw run done", file=sys.stderr)
    _LAST["exec_time_ns"] = res.exec_time_ns
    _LAST["results"] = res
    out = np.empty((N, DIM), np.float32)
    for c in range(N_CORES):
        r = res.results[c]
        acc = np.asarray(r["out"])[:N_LOC, :DIM].astype(np.float32)
        for i in range(N_TBL):
            acc = acc + np.asarray(r[f"tbl{i}"])[:N_LOC, :DIM].astype(np.float32)
        out[order[c * N_LOC : (c + 1) * N_LOC]] = acc
    return out
